# revision 55
# baseline (speedup 1.0000x reference)
"""Trainium2 Bass kernel for nn_DeformConv2d (B=16, Cin=Cout=64, H=W=64, K=3).

Strategy (data-parallel over batch, 2 images per core on 8 cores):
  1. PE: offset conv (9 accumulating matmuls per image, K=64, M=18),
     processed in two 2048-column halves.
  2. ACT: bilinear "tent" coefficients tent(delta - D) = relu(1 - |delta - D|)
     via Abs/Relu activations in a compact layout amap[img*9 + j, n] per tap
     (j = window offset); DVE multiplies the y/x tent factors.
  3. Hybrid broadcast of the per-(tap, j) coefficient row pair across the
     128 channel partitions:
       - js {0,1,8}: SWDGE DMA (gpsimd-issued; spreads over all 16 SDMA
         engines) replicating from a DRAM copy of the coefficient maps.
       - js {2..7}: PE ones-mask matmul ([18,128] selector lhsT) into PSUM,
         then ACT copies PSUM -> SBUF fp16.
     This splits the replication load across three otherwise-idle paths so
     the DVE multiply-accumulate stays the critical path.
  4. DVE: 81-term shifted-window multiply-accumulate builds the im2col
     tensor cols per tap (9 mults + 8 adds of [128, 2048] fp16 per tap/half).
  5. PE: main conv = 9 accumulating matmuls (K=64, M=64) per image per half
     into a [128, 2048] f32 PSUM strip; ACT adds bias and writes f32 out.

On-chip compute is fp16 (DVE 2x mode; PSUM accumulates in f32).
kernel() accepts FULL inputs and returns the FULL [16,64,64,64] output.
"""

import numpy as np
from contextlib import ExitStack

N_CORES = 8
B, CIN, COUT, H, W = 16, 64, 64, 64, 64
KK = 9  # 3x3 taps
HW = H * W  # 4096
PADR, PADC = 2, 2
HP, WP = H + 2 * PADR, W + 2 * PADC  # 68, 68
IMG_PER_CORE = B // N_CORES  # 2
HALF = HW // 2

DMA_JS = (0, 1)  # js broadcast via SWDGE DMA (as one pair)
_cache = {}
DEBUG = False


def _build_program():
    import concourse.bass as bass  # noqa: F401
    import concourse.mybir as mybir
    import concourse.tile as tile
    from concourse import bacc

    fp16 = mybir.dt.float16
    f32 = mybir.dt.float32
    AOp = mybir.AluOpType
    AF = mybir.ActivationFunctionType

    nc = bacc.Bacc("TRN2", target_bir_lowering=False, debug=False,
                   num_devices=N_CORES)

    xp_ext = nc.declare_dram_parameter("xp", [128, HP * WP], fp16, isOutput=False)
    woff_ext = nc.declare_dram_parameter("woff", [KK, CIN, 18], fp16, isOutput=False)
    wdcn_ext = nc.declare_dram_parameter("wdcn", [KK, CIN, COUT], fp16, isOutput=False)
    boff_ext = nc.declare_dram_parameter("boff", [64, 1], f32, isOutput=False)
    bdcn_ext = nc.declare_dram_parameter("bdcn", [128, 1], f32, isOutput=False)
    ones2_ext = nc.declare_dram_parameter("ones2", [18, KK * 128], fp16, isOutput=False)
    # per-row tent window offsets: biases -D for |delta - D|
    dyb_ext = nc.declare_dram_parameter("dyb", [18, 1], f32, isOutput=False)
    dxb_ext = nc.declare_dram_parameter("dxb", [18, 1], f32, isOutput=False)
    one18_ext = nc.declare_dram_parameter("one18", [18, 1], f32, isOutput=False)
    out_ext = nc.declare_dram_parameter("out", [128, HW], f32, isOutput=True)

    offs_dram = nc.dram_tensor("offs_dram", [64, HW], fp16)
    # rows = img; cols = (kk, half, j, n) so j-runs are contiguous per (kk, h)
    amap_dram = nc.dram_tensor("amap_dram", [2, KK * 2 * KK * HALF], fp16)

    with tile.TileContext(nc) as tc, ExitStack() as ctx:
        pool = ctx.enter_context(tc.tile_pool(name="sbuf", bufs=1))
        apool = ctx.enter_context(tc.tile_pool(name="amaps", bufs=1))
        tpool = ctx.enter_context(tc.tile_pool(name="tents", bufs=1))
        abuf = ctx.enter_context(tc.tile_pool(name="areps", bufs=3))
        dbuf = ctx.enter_context(tc.tile_pool(name="dstream", bufs=2))
        ppool = ctx.enter_context(tc.tile_pool(name="psum", bufs=1, space="PSUM"))
        pbc = ctx.enter_context(tc.tile_pool(name="psumbc", bufs=2, space="PSUM"))

        # ---- inputs ----
        xp = pool.tile([128, HP * WP], fp16)
        nc.gpsimd.dma_start(xp[:], xp_ext[:])
        xp3 = xp[:].rearrange("p (r c) -> p r c", c=WP)  # [128, 68, 68]

        # weights live on BOTH partition halves (matmul lhsT must share the
        # rhs base partition; img1 rhs starts at partition 64)
        woff = pool.tile([128, KK * 18], fp16)
        wdcn = pool.tile([128, KK * COUT], fp16)
        for hh in range(2):
            nc.sync.dma_start(
                woff[hh * 64 : (hh + 1) * 64, :].rearrange("c (k m) -> c k m", m=18),
                woff_ext[:].rearrange("k c m -> c k m"),
            )
            nc.sync.dma_start(
                wdcn[hh * 64 : (hh + 1) * 64, :].rearrange("c (k m) -> c k m", m=COUT),
                wdcn_ext[:].rearrange("k c m -> c k m"),
            )
        boff = pool.tile([64, 1], f32)
        nc.sync.dma_start(boff[:], boff_ext[:])
        bdcn = pool.tile([128, 1], f32)
        nc.sync.dma_start(bdcn[:], bdcn_ext[:])
        ones2 = pool.tile([18, KK * 128], fp16)
        nc.sync.dma_start(ones2[:], ones2_ext[:])
        dyb = pool.tile([18, 1], f32)
        nc.sync.dma_start(dyb[:], dyb_ext[:])
        dxb = pool.tile([18, 1], f32)
        nc.sync.dma_start(dxb[:], dxb_ext[:])
        one18 = pool.tile([18, 1], f32)
        nc.sync.dma_start(one18[:], one18_ext[:])

        offs_sb = pool.tile([64, HW], fp16)
        out_sb = pool.tile([128, HW], f32)

        # ---- S1/S2: offset conv in 1024-col chunks on rotating PSUM tiles
        # (bc pool), so h1 chunks can be emitted during the h0 MAC ----
        def emit_offconv_chunk(q):
            ps = pbc.tile([128, 1024], f32, tag="bc")
            for img in range(IMG_PER_CORE):
                for sub in range(2):
                    tg = q * 2 + sub
                    for kk in range(KK):
                        ky, kx = kk // 3, kk % 3
                        rhs = xp3[
                            img * 64 : (img + 1) * 64,
                            (PADR - 1 + ky + 8 * tg) : (PADR - 1 + ky + 8 * tg + 8),
                            (PADC - 1 + kx) : (PADC - 1 + kx + W),
                        ]
                        nc.tensor.matmul(
                            ps[img * 32 : img * 32 + 18, sub * 512 : (sub + 1) * 512],
                            woff[img * 64 : (img + 1) * 64, kk * 18 : (kk + 1) * 18],
                            rhs,
                            start=(kk == 0),
                            stop=(kk == KK - 1),
                        )
            qs = slice(q * 1024, (q + 1) * 1024)
            nc.scalar.activation(
                out=offs_sb[:, qs], in_=ps[0:64, :],
                func=AF.Identity, bias=boff[:],
            )
            nc.sync.dma_start(offs_dram[:, qs], offs_sb[:, qs])

        emit_offconv_chunk(0)
        emit_offconv_chunk(1)

        # offs_dram rows = img*32 + 2*kk + axis
        offs2 = offs_dram[:].rearrange("(i m) n -> i m n", i=2)  # [2, 32, HW]
        # amap_dram views: [2, kk, h, j, n] and [2, kk, h, j*n]
        ad5 = amap_dram[:].rearrange(
            "p (k h j n) -> p k h j n", k=KK, h=2, j=KK
        )
        ad4 = amap_dram[:].rearrange(
            "p (k h m) -> p k h m", k=KK, h=2
        )  # [2, 9, 2, KK*HALF]

        # ---- main loop: tents -> hybrid broadcast -> DVE MAC -> matmuls ----
        amap_tiles = {}

        def emit_tents_c(kk, c):
            # tents for tap kk, compact layout [18, HALF] per half: row img*9+j
            if True:
                amap_t = apool.tile([18, HALF], fp16, tag=f"amap{kk}_{c}")
                amap_tiles[(kk, c)] = amap_t
                cs = slice(c * HALF, (c + 1) * HALF)
                tin_y = tpool.tile([18, HALF], fp16, tag="tiny")
                src = offs2[:, 2 * kk : 2 * kk + 1, cs].broadcast_to(
                    [2, KK, HALF]
                )
                nc.sync.dma_start(tin_y[:], src)
                tin_x = tpool.tile([18, HALF], fp16, tag="tinx")
                src = offs2[:, 2 * kk + 1 : 2 * kk + 2, cs].broadcast_to(
                    [2, KK, HALF]
                )
                nc.sync.dma_start(tin_x[:], src)
                uy = tpool.tile([18, HALF], fp16, tag="uy")
                nc.scalar.activation(
                    out=uy[:], in_=tin_y[:], func=AF.Abs, bias=dyb[:]
                )
                ux = tpool.tile([18, HALF], fp16, tag="ux")
                nc.scalar.activation(
                    out=ux[:], in_=tin_x[:], func=AF.Abs, bias=dxb[:]
                )
                # min(u,1) - 1 = -relu(1-u); the negations cancel in the
                # product, so amap = relu(1-uy) * relu(1-ux) exactly.
                ty = tpool.tile([18, HALF], fp16, tag="ty")
                nc.vector.tensor_scalar(
                    out=ty[:], in0=uy[:], scalar1=1.0, scalar2=1.0,
                    op0=AOp.min, op1=AOp.subtract,
                )
                tx = tpool.tile([18, HALF], fp16, tag="tx")
                nc.vector.tensor_scalar(
                    out=tx[:], in0=ux[:], scalar1=1.0, scalar2=1.0,
                    op0=AOp.min, op1=AOp.subtract,
                )
                nc.gpsimd.tensor_tensor(
                    out=amap_t[:], in0=ty[:], in1=tx[:], op=AOp.mult
                )
                for j in DMA_JS:
                    nc.sync.dma_start(
                        ad5[:, kk, c, j, :],
                        amap_t[j : j + KK + 1 : KK, :],
                    )

        emit_tents_c(0, 0)
        for h in range(2):
            ps_main = ppool.tile([128, HALF], f32, tag="big")
            for kk in range(KK):
                ky, kx = kk // 3, kk % 3
                amap_t = amap_tiles[(kk, h)]

                # SWDGE broadcast for js {0,1} as one pair (issued at block
                # head so it is not queued behind the gpsimd tensor ops)
                pairt = abuf.tile([128, 2 * HALF], fp16, tag="pair")
                src = (
                    ad4[:, kk, h, 0 : 2 * HALF]
                    .unsqueeze(1)
                    .broadcast_to([2, 64, 2 * HALF])
                )
                nc.gpsimd.dma_start(pairt[:], src)
                if h == 0:
                    if kk + 1 < KK:
                        emit_tents_c(kk + 1, 0)
                    if kk < 2:
                        emit_offconv_chunk(2 + kk)
                    if kk >= 1:
                        emit_tents_c(kk - 1, 1)
                if h == 1 and kk == 0:
                    emit_tents_c(7, 1)
                    emit_tents_c(8, 1)

                cols = dbuf.tile([128, HALF], fp16, tag="cols")
                for ji, j in enumerate((2, 3, 4, 5, 6, 7, 8, 0, 1)):
                    dy, dx = j // 3 - 1, j % 3 - 1
                    if j in (0, 1):
                        arep_ap = pairt[:, j * HALF : (j + 1) * HALF]
                    else:
                        # PE ones-mask broadcast into PSUM + ACT copy
                        arep = dbuf.tile([128, HALF], fp16, tag="arep")
                        for q in range(2):
                            psb = pbc.tile([128, 1024], f32, tag="bc")
                            for cc in range(2):
                                n0 = q * 1024 + cc * 512
                                nc.tensor.matmul(
                                    psb[:, cc * 512 : (cc + 1) * 512],
                                    ones2[:, j * 128 : (j + 1) * 128],
                                    amap_t[:, n0 : n0 + 512],
                                    start=True,
                                    stop=True,
                                )
                            nc.scalar.activation(
                                out=arep[:, q * 1024 : (q + 1) * 1024],
                                in_=psb[:], func=AF.Identity,
                            )
                        arep_ap = arep[:]
                    xwin = xp3[
                        :,
                        (PADR - 1 + ky + dy + 32 * h) : (PADR - 1 + ky + dy + 32 * h + 32),
                        (PADC - 1 + kx + dx) : (PADC - 1 + kx + dx + W),
                    ]  # [128, 32, 64]
                    if ji == 0:
                        nc.vector.tensor_tensor(
                            out=cols[:].rearrange("p (a b) -> p a b", b=W),
                            in0=xwin,
                            in1=arep_ap.rearrange("p (a b) -> p a b", b=W),
                            op=AOp.mult,
                        )
                    else:
                        prod = dbuf.tile([128, HALF], fp16, tag="prod")
                        nc.vector.tensor_tensor(
                            out=prod[:].rearrange("p (a b) -> p a b", b=W),
                            in0=xwin,
                            in1=arep_ap.rearrange("p (a b) -> p a b", b=W),
                            op=AOp.mult,
                        )
                        nc.vector.tensor_tensor(
                            out=cols[:], in0=cols[:], in1=prod[:], op=AOp.add
                        )
                for img in range(IMG_PER_CORE):
                    for t in range(4):
                        nc.tensor.matmul(
                            ps_main[
                                img * 64 : (img + 1) * 64, t * 512 : (t + 1) * 512
                            ],
                            wdcn[img * 64 : (img + 1) * 64, kk * COUT : (kk + 1) * COUT],
                            cols[img * 64 : (img + 1) * 64, t * 512 : (t + 1) * 512],
                            start=(kk == 0),
                            stop=(kk == KK - 1),
                        )
            hs = slice(h * HALF, (h + 1) * HALF)
            nc.scalar.activation(
                out=out_sb[:, hs], in_=ps_main[:],
                func=AF.Identity, bias=bdcn[:],
            )
            nc.sync.dma_start(out_ext[:, hs], out_sb[:, hs])

    nc.compile()
    return nc


def _host_prep(x, w_off, b_off, w_dcn, b_dcn):
    """Per-core input maps. numpy layout/dtype prep only."""
    fp16 = np.float16
    x = np.asarray(x, dtype=np.float32)
    w_off = np.asarray(w_off, dtype=np.float32)
    b_off = np.asarray(b_off, dtype=np.float32)
    w_dcn = np.asarray(w_dcn, dtype=np.float32)
    b_dcn = np.asarray(b_dcn, dtype=np.float32)

    # lhsT per tap: [KK, CIN, M]
    woff_l = np.ascontiguousarray(
        w_off.transpose(2, 3, 1, 0).reshape(KK, CIN, 18)
    ).astype(fp16)
    wdcn_l = np.ascontiguousarray(
        w_dcn.transpose(2, 3, 1, 0).reshape(KK, CIN, COUT)
    ).astype(fp16)

    boff_rep = np.zeros((64, 1), np.float32)
    for img in range(IMG_PER_CORE):
        boff_rep[img * 32 : img * 32 + 18, 0] = b_off
    bdcn_rep = np.tile(b_dcn, IMG_PER_CORE).reshape(128, 1).astype(np.float32)

    ones2 = np.zeros((18, KK * 128), fp16)
    dyb = np.zeros((18, 1), np.float32)
    dxb = np.zeros((18, 1), np.float32)
    for j in range(KK):
        for img in range(2):
            r = img * KK + j
            ones2[r, j * 128 + img * 64 : j * 128 + (img + 1) * 64] = 1.0
            dyb[r, 0] = -(j // 3 - 1)
            dxb[r, 0] = -(j % 3 - 1)
    one18 = np.ones((18, 1), np.float32)

    shared = {
        "woff": woff_l,
        "wdcn": wdcn_l,
        "boff": boff_rep,
        "bdcn": bdcn_rep,
        "ones2": ones2,
        "dyb": dyb,
        "dxb": dxb,
        "one18": one18,
    }
    in_maps = []
    for core in range(N_CORES):
        imgs = x[core * IMG_PER_CORE : (core + 1) * IMG_PER_CORE]
        xp = np.zeros((IMG_PER_CORE, CIN, HP, WP), np.float32)
        xp[:, :, PADR : PADR + H, PADC : PADC + W] = imgs
        m = {"xp": xp.reshape(128, HP * WP).astype(fp16)}
        m.update(shared)
        in_maps.append(m)
    return in_maps


def kernel(x, w_off, b_off, w_dcn, b_dcn, _trace=False):
    from concourse.bass_utils import run_bass_kernel_spmd

    if "nc" not in _cache:
        _cache["nc"] = _build_program()
    nc = _cache["nc"]

    in_maps = _host_prep(x, w_off, b_off, w_dcn, b_dcn)
    res = run_bass_kernel_spmd(nc, in_maps, list(range(N_CORES)), trace=_trace)
    _cache["last_result"] = res

    out = np.empty((B, COUT, H, W), np.float32)
    for core in range(N_CORES):
        o = np.asarray(res.results[core]["out"], dtype=np.float32)
        out[core * IMG_PER_CORE : (core + 1) * IMG_PER_CORE] = o.reshape(
            IMG_PER_CORE, COUT, H, W
        )
    return out


# revision 56
# speedup vs baseline: 1.0824x; 1.0824x over previous
"""Trainium2 Bass kernel for nn_DeformConv2d (B=16, Cin=Cout=64, H=W=64, K=3).

Strategy (data-parallel over batch, 2 images per core on 8 cores):
  1. PE: offset conv (9 accumulating matmuls per image, K=64, M=18),
     processed in two 2048-column halves.
  2. ACT: bilinear "tent" coefficients tent(delta - D) = relu(1 - |delta - D|)
     via Abs/Relu activations in a compact layout amap[img*9 + j, n] per tap
     (j = window offset); DVE multiplies the y/x tent factors.
  3. Hybrid broadcast of the per-(tap, j) coefficient row pair across the
     128 channel partitions:
       - js {0,1,8}: SWDGE DMA (gpsimd-issued; spreads over all 16 SDMA
         engines) replicating from a DRAM copy of the coefficient maps.
       - js {2..7}: PE ones-mask matmul ([18,128] selector lhsT) into PSUM,
         then ACT copies PSUM -> SBUF fp16.
     This splits the replication load across three otherwise-idle paths so
     the DVE multiply-accumulate stays the critical path.
  4. DVE: 81-term shifted-window multiply-accumulate builds the im2col
     tensor cols per tap (9 mults + 8 adds of [128, 2048] fp16 per tap/half).
  5. PE: main conv = 9 accumulating matmuls (K=64, M=64) per image per half
     into a [128, 2048] f32 PSUM strip; ACT adds bias and writes f32 out.

On-chip compute is fp16 (DVE 2x mode; PSUM accumulates in f32).
kernel() accepts FULL inputs and returns the FULL [16,64,64,64] output.
"""

import numpy as np
from contextlib import ExitStack

N_CORES = 8
B, CIN, COUT, H, W = 16, 64, 64, 64, 64
KK = 9  # 3x3 taps
HW = H * W  # 4096
PADR, PADC = 2, 2
HP, WP = H + 2 * PADR, W + 2 * PADC  # 68, 68
IMG_PER_CORE = B // N_CORES  # 2
HALF = HW // 2

DMA_JS = (0, 1)  # js broadcast via SWDGE DMA (as one pair)
_cache = {}
DEBUG = False


def _build_program():
    import concourse.bass as bass  # noqa: F401
    import concourse.mybir as mybir
    import concourse.tile as tile
    from concourse import bacc

    fp16 = mybir.dt.float16
    f32 = mybir.dt.float32
    AOp = mybir.AluOpType
    AF = mybir.ActivationFunctionType

    nc = bacc.Bacc("TRN2", target_bir_lowering=False, debug=False,
                   num_devices=N_CORES)

    xp_ext = nc.declare_dram_parameter("xp", [128, HP * WP], fp16, isOutput=False)
    woff_ext = nc.declare_dram_parameter("woff", [KK, CIN, 18], fp16, isOutput=False)
    wdcn_ext = nc.declare_dram_parameter("wdcn", [KK, CIN, COUT], fp16, isOutput=False)
    boff_ext = nc.declare_dram_parameter("boff", [64, 1], f32, isOutput=False)
    bdcn_ext = nc.declare_dram_parameter("bdcn", [128, 1], f32, isOutput=False)
    ones2_ext = nc.declare_dram_parameter("ones2", [18, KK * 128], fp16, isOutput=False)
    # per-row tent window offsets: biases -D for |delta - D|
    dyb_ext = nc.declare_dram_parameter("dyb", [18, 1], f32, isOutput=False)
    dxb_ext = nc.declare_dram_parameter("dxb", [18, 1], f32, isOutput=False)
    one18_ext = nc.declare_dram_parameter("one18", [18, 1], f32, isOutput=False)
    out_ext = nc.declare_dram_parameter("out", [128, HW], f32, isOutput=True)

    offs_dram = nc.dram_tensor("offs_dram", [64, HW], fp16)
    # rows = img; cols = (kk, half, j, n) so j-runs are contiguous per (kk, h)
    amap_dram = nc.dram_tensor("amap_dram", [2, KK * 2 * KK * HALF], fp16)

    with tile.TileContext(nc) as tc, ExitStack() as ctx:
        pool = ctx.enter_context(tc.tile_pool(name="sbuf", bufs=1))
        apool = ctx.enter_context(tc.tile_pool(name="amaps", bufs=1))
        tpool = ctx.enter_context(tc.tile_pool(name="tents", bufs=1))
        abuf = ctx.enter_context(tc.tile_pool(name="areps", bufs=3))
        dbuf = ctx.enter_context(tc.tile_pool(name="dstream", bufs=2))
        ppool = ctx.enter_context(tc.tile_pool(name="psum", bufs=1, space="PSUM"))
        pbc = ctx.enter_context(tc.tile_pool(name="psumbc", bufs=2, space="PSUM"))

        # ---- inputs ----
        xp = pool.tile([128, HP * WP], fp16)
        nc.gpsimd.dma_start(xp[:], xp_ext[:])
        xp3 = xp[:].rearrange("p (r c) -> p r c", c=WP)  # [128, 68, 68]

        # weights live on BOTH partition halves (matmul lhsT must share the
        # rhs base partition; img1 rhs starts at partition 64)
        woff = pool.tile([128, KK * 18], fp16)
        wdcn = pool.tile([128, KK * COUT], fp16)
        for hh in range(2):
            nc.sync.dma_start(
                woff[hh * 64 : (hh + 1) * 64, :].rearrange("c (k m) -> c k m", m=18),
                woff_ext[:].rearrange("k c m -> c k m"),
            )
            nc.sync.dma_start(
                wdcn[hh * 64 : (hh + 1) * 64, :].rearrange("c (k m) -> c k m", m=COUT),
                wdcn_ext[:].rearrange("k c m -> c k m"),
            )
        boff = pool.tile([64, 1], f32)
        nc.sync.dma_start(boff[:], boff_ext[:])
        bdcn = pool.tile([128, 1], f32)
        nc.sync.dma_start(bdcn[:], bdcn_ext[:])
        ones2 = pool.tile([18, KK * 128], fp16)
        nc.sync.dma_start(ones2[:], ones2_ext[:])
        dyb = pool.tile([18, 1], f32)
        nc.sync.dma_start(dyb[:], dyb_ext[:])
        dxb = pool.tile([18, 1], f32)
        nc.sync.dma_start(dxb[:], dxb_ext[:])
        one18 = pool.tile([18, 1], f32)
        nc.sync.dma_start(one18[:], one18_ext[:])

        offs_sb = pool.tile([64, HW], fp16)
        out_sb = pool.tile([128, HW], f32)

        # ---- S1/S2: offset conv in 1024-col chunks on rotating PSUM tiles
        # (bc pool), so h1 chunks can be emitted during the h0 MAC ----
        def emit_offconv_chunk(q):
            ps = pbc.tile([128, 1024], f32, tag="bc")
            for img in range(IMG_PER_CORE):
                for sub in range(2):
                    tg = q * 2 + sub
                    for kk in range(KK):
                        ky, kx = kk // 3, kk % 3
                        rhs = xp3[
                            img * 64 : (img + 1) * 64,
                            (PADR - 1 + ky + 8 * tg) : (PADR - 1 + ky + 8 * tg + 8),
                            (PADC - 1 + kx) : (PADC - 1 + kx + W),
                        ]
                        nc.tensor.matmul(
                            ps[img * 32 : img * 32 + 18, sub * 512 : (sub + 1) * 512],
                            woff[img * 64 : (img + 1) * 64, kk * 18 : (kk + 1) * 18],
                            rhs,
                            start=(kk == 0),
                            stop=(kk == KK - 1),
                        )
            qs = slice(q * 1024, (q + 1) * 1024)
            nc.scalar.activation(
                out=offs_sb[:, qs], in_=ps[0:64, :],
                func=AF.Identity, bias=boff[:],
            )
            nc.sync.dma_start(offs_dram[:, qs], offs_sb[:, qs])

        emit_offconv_chunk(0)
        emit_offconv_chunk(1)

        # offs_dram rows = img*32 + 2*kk + axis
        offs2 = offs_dram[:].rearrange("(i m) n -> i m n", i=2)  # [2, 32, HW]
        # amap_dram views: [2, kk, h, j, n] and [2, kk, h, j*n]
        ad5 = amap_dram[:].rearrange(
            "p (k h j n) -> p k h j n", k=KK, h=2, j=KK
        )
        ad4 = amap_dram[:].rearrange(
            "p (k h m) -> p k h m", k=KK, h=2
        )  # [2, 9, 2, KK*HALF]

        # ---- main loop: tents -> hybrid broadcast -> DVE MAC -> matmuls ----
        amap_tiles = {}

        def emit_tents_c(kk, c):
            # tents for tap kk, compact layout [18, HALF] per half: row img*9+j
            if True:
                amap_t = apool.tile([18, HALF], fp16, tag=f"amap{kk}_{c}")
                amap_tiles[(kk, c)] = amap_t
                cs = slice(c * HALF, (c + 1) * HALF)
                tin_y = tpool.tile([18, HALF], fp16, tag="tiny")
                src = offs2[:, 2 * kk : 2 * kk + 1, cs].broadcast_to(
                    [2, KK, HALF]
                )
                nc.sync.dma_start(tin_y[:], src)
                tin_x = tpool.tile([18, HALF], fp16, tag="tinx")
                src = offs2[:, 2 * kk + 1 : 2 * kk + 2, cs].broadcast_to(
                    [2, KK, HALF]
                )
                nc.sync.dma_start(tin_x[:], src)
                uy = tpool.tile([18, HALF], fp16, tag="uy")
                nc.scalar.activation(
                    out=uy[:], in_=tin_y[:], func=AF.Abs, bias=dyb[:]
                )
                ux = tpool.tile([18, HALF], fp16, tag="ux")
                nc.scalar.activation(
                    out=ux[:], in_=tin_x[:], func=AF.Abs, bias=dxb[:]
                )
                # min(u,1) - 1 = -relu(1-u); the negations cancel in the
                # product, so amap = relu(1-uy) * relu(1-ux) exactly.
                ty = tpool.tile([18, HALF], fp16, tag="ty")
                nc.vector.tensor_scalar(
                    out=ty[:], in0=uy[:], scalar1=1.0, scalar2=1.0,
                    op0=AOp.min, op1=AOp.subtract,
                )
                tx = tpool.tile([18, HALF], fp16, tag="tx")
                nc.vector.tensor_scalar(
                    out=tx[:], in0=ux[:], scalar1=1.0, scalar2=1.0,
                    op0=AOp.min, op1=AOp.subtract,
                )
                nc.vector.tensor_tensor(
                    out=amap_t[:], in0=ty[:], in1=tx[:], op=AOp.mult
                )
                for j in DMA_JS:
                    nc.sync.dma_start(
                        ad5[:, kk, c, j, :],
                        amap_t[j : j + KK + 1 : KK, :],
                    )

        emit_tents_c(0, 0)
        for h in range(2):
            ps_main = ppool.tile([128, HALF], f32, tag="big")
            for kk in range(KK):
                ky, kx = kk // 3, kk % 3
                amap_t = amap_tiles[(kk, h)]

                # SWDGE broadcast for js {0,1} as one pair (issued at block
                # head so it is not queued behind the gpsimd tensor ops)
                pairt = abuf.tile([128, 2 * HALF], fp16, tag="pair")
                src = (
                    ad4[:, kk, h, 0 : 2 * HALF]
                    .unsqueeze(1)
                    .broadcast_to([2, 64, 2 * HALF])
                )
                nc.gpsimd.dma_start(pairt[:], src)
                if h == 0:
                    if kk + 1 < KK:
                        emit_tents_c(kk + 1, 0)
                    if kk < 2:
                        emit_offconv_chunk(2 + kk)
                    if kk >= 1:
                        emit_tents_c(kk - 1, 1)
                if h == 1 and kk == 0:
                    emit_tents_c(7, 1)
                    emit_tents_c(8, 1)

                cols = dbuf.tile([128, HALF], fp16, tag="cols")
                for ji, j in enumerate((2, 3, 4, 5, 6, 7, 8, 0, 1)):
                    dy, dx = j // 3 - 1, j % 3 - 1
                    if j in (0, 1):
                        arep_ap = pairt[:, j * HALF : (j + 1) * HALF]
                    else:
                        # PE ones-mask broadcast into PSUM + ACT copy
                        arep = dbuf.tile([128, HALF], fp16, tag="arep")
                        for q in range(2):
                            psb = pbc.tile([128, 1024], f32, tag="bc")
                            for cc in range(2):
                                n0 = q * 1024 + cc * 512
                                nc.tensor.matmul(
                                    psb[:, cc * 512 : (cc + 1) * 512],
                                    ones2[:, j * 128 : (j + 1) * 128],
                                    amap_t[:, n0 : n0 + 512],
                                    start=True,
                                    stop=True,
                                )
                            nc.scalar.activation(
                                out=arep[:, q * 1024 : (q + 1) * 1024],
                                in_=psb[:], func=AF.Identity,
                            )
                        arep_ap = arep[:]
                    xwin = xp3[
                        :,
                        (PADR - 1 + ky + dy + 32 * h) : (PADR - 1 + ky + dy + 32 * h + 32),
                        (PADC - 1 + kx + dx) : (PADC - 1 + kx + dx + W),
                    ]  # [128, 32, 64]
                    if ji == 0:
                        nc.vector.tensor_tensor(
                            out=cols[:].rearrange("p (a b) -> p a b", b=W),
                            in0=xwin,
                            in1=arep_ap.rearrange("p (a b) -> p a b", b=W),
                            op=AOp.mult,
                        )
                    else:
                        prod = dbuf.tile([128, HALF], fp16, tag="prod")
                        nc.vector.tensor_tensor(
                            out=prod[:].rearrange("p (a b) -> p a b", b=W),
                            in0=xwin,
                            in1=arep_ap.rearrange("p (a b) -> p a b", b=W),
                            op=AOp.mult,
                        )
                        nc.vector.tensor_tensor(
                            out=cols[:], in0=cols[:], in1=prod[:], op=AOp.add
                        )
                for img in range(IMG_PER_CORE):
                    for t in range(4):
                        nc.tensor.matmul(
                            ps_main[
                                img * 64 : (img + 1) * 64, t * 512 : (t + 1) * 512
                            ],
                            wdcn[img * 64 : (img + 1) * 64, kk * COUT : (kk + 1) * COUT],
                            cols[img * 64 : (img + 1) * 64, t * 512 : (t + 1) * 512],
                            start=(kk == 0),
                            stop=(kk == KK - 1),
                        )
            hs = slice(h * HALF, (h + 1) * HALF)
            nc.scalar.activation(
                out=out_sb[:, hs], in_=ps_main[:],
                func=AF.Identity, bias=bdcn[:],
            )
            nc.sync.dma_start(out_ext[:, hs], out_sb[:, hs])

    nc.compile()
    return nc


def _host_prep(x, w_off, b_off, w_dcn, b_dcn):
    """Per-core input maps. numpy layout/dtype prep only."""
    fp16 = np.float16
    x = np.asarray(x, dtype=np.float32)
    w_off = np.asarray(w_off, dtype=np.float32)
    b_off = np.asarray(b_off, dtype=np.float32)
    w_dcn = np.asarray(w_dcn, dtype=np.float32)
    b_dcn = np.asarray(b_dcn, dtype=np.float32)

    # lhsT per tap: [KK, CIN, M]
    woff_l = np.ascontiguousarray(
        w_off.transpose(2, 3, 1, 0).reshape(KK, CIN, 18)
    ).astype(fp16)
    wdcn_l = np.ascontiguousarray(
        w_dcn.transpose(2, 3, 1, 0).reshape(KK, CIN, COUT)
    ).astype(fp16)

    boff_rep = np.zeros((64, 1), np.float32)
    for img in range(IMG_PER_CORE):
        boff_rep[img * 32 : img * 32 + 18, 0] = b_off
    bdcn_rep = np.tile(b_dcn, IMG_PER_CORE).reshape(128, 1).astype(np.float32)

    ones2 = np.zeros((18, KK * 128), fp16)
    dyb = np.zeros((18, 1), np.float32)
    dxb = np.zeros((18, 1), np.float32)
    for j in range(KK):
        for img in range(2):
            r = img * KK + j
            ones2[r, j * 128 + img * 64 : j * 128 + (img + 1) * 64] = 1.0
            dyb[r, 0] = -(j // 3 - 1)
            dxb[r, 0] = -(j % 3 - 1)
    one18 = np.ones((18, 1), np.float32)

    shared = {
        "woff": woff_l,
        "wdcn": wdcn_l,
        "boff": boff_rep,
        "bdcn": bdcn_rep,
        "ones2": ones2,
        "dyb": dyb,
        "dxb": dxb,
        "one18": one18,
    }
    in_maps = []
    for core in range(N_CORES):
        imgs = x[core * IMG_PER_CORE : (core + 1) * IMG_PER_CORE]
        xp = np.zeros((IMG_PER_CORE, CIN, HP, WP), np.float32)
        xp[:, :, PADR : PADR + H, PADC : PADC + W] = imgs
        m = {"xp": xp.reshape(128, HP * WP).astype(fp16)}
        m.update(shared)
        in_maps.append(m)
    return in_maps


def kernel(x, w_off, b_off, w_dcn, b_dcn, _trace=False):
    from concourse.bass_utils import run_bass_kernel_spmd

    if "nc" not in _cache:
        _cache["nc"] = _build_program()
    nc = _cache["nc"]

    in_maps = _host_prep(x, w_off, b_off, w_dcn, b_dcn)
    res = run_bass_kernel_spmd(nc, in_maps, list(range(N_CORES)), trace=_trace)
    _cache["last_result"] = res

    out = np.empty((B, COUT, H, W), np.float32)
    for core in range(N_CORES):
        o = np.asarray(res.results[core]["out"], dtype=np.float32)
        out[core * IMG_PER_CORE : (core + 1) * IMG_PER_CORE] = o.reshape(
            IMG_PER_CORE, COUT, H, W
        )
    return out


# revision 58
# speedup vs baseline: 1.0907x; 1.0076x over previous
"""Trainium2 Bass kernel for nn_DeformConv2d (B=16, Cin=Cout=64, H=W=64, K=3).

Strategy (data-parallel over batch, 2 images per core on 8 cores):
  1. PE: offset conv (9 accumulating matmuls per image, K=64, M=18),
     processed in two 2048-column halves.
  2. ACT: bilinear "tent" coefficients tent(delta - D) = relu(1 - |delta - D|)
     via Abs/Relu activations in a compact layout amap[img*9 + j, n] per tap
     (j = window offset); DVE multiplies the y/x tent factors.
  3. Hybrid broadcast of the per-(tap, j) coefficient row pair across the
     128 channel partitions:
       - js {0,1,8}: SWDGE DMA (gpsimd-issued; spreads over all 16 SDMA
         engines) replicating from a DRAM copy of the coefficient maps.
       - js {2..7}: PE ones-mask matmul ([18,128] selector lhsT) into PSUM,
         then ACT copies PSUM -> SBUF fp16.
     This splits the replication load across three otherwise-idle paths so
     the DVE multiply-accumulate stays the critical path.
  4. DVE: 81-term shifted-window multiply-accumulate builds the im2col
     tensor cols per tap (9 mults + 8 adds of [128, 2048] fp16 per tap/half).
  5. PE: main conv = 9 accumulating matmuls (K=64, M=64) per image per half
     into a [128, 2048] f32 PSUM strip; ACT adds bias and writes f32 out.

On-chip compute is fp16 (DVE 2x mode; PSUM accumulates in f32).
kernel() accepts FULL inputs and returns the FULL [16,64,64,64] output.
"""

import numpy as np
from contextlib import ExitStack

N_CORES = 8
B, CIN, COUT, H, W = 16, 64, 64, 64, 64
KK = 9  # 3x3 taps
HW = H * W  # 4096
PADR, PADC = 2, 2
HP, WP = H + 2 * PADR, W + 2 * PADC  # 68, 68
IMG_PER_CORE = B // N_CORES  # 2
HALF = HW // 2

DMA_JS = (0, 1)  # js broadcast via SWDGE DMA (as one pair)
_cache = {}
DEBUG = False


def _build_program():
    import concourse.bass as bass  # noqa: F401
    import concourse.mybir as mybir
    import concourse.tile as tile
    from concourse import bacc

    fp16 = mybir.dt.float16
    f32 = mybir.dt.float32
    AOp = mybir.AluOpType
    AF = mybir.ActivationFunctionType

    nc = bacc.Bacc("TRN2", target_bir_lowering=False, debug=False,
                   num_devices=N_CORES)

    xp_ext = nc.declare_dram_parameter("xp", [128, HP * WP], fp16, isOutput=False)
    woff_ext = nc.declare_dram_parameter("woff", [KK, CIN, 18], fp16, isOutput=False)
    wdcn_ext = nc.declare_dram_parameter("wdcn", [KK, CIN, COUT], fp16, isOutput=False)
    boff_ext = nc.declare_dram_parameter("boff", [64, 1], f32, isOutput=False)
    bdcn_ext = nc.declare_dram_parameter("bdcn", [128, 1], f32, isOutput=False)
    ones2_ext = nc.declare_dram_parameter("ones2", [18, KK * 128], fp16, isOutput=False)
    # per-row tent window offsets: biases -D for |delta - D|
    dyb_ext = nc.declare_dram_parameter("dyb", [18, 1], f32, isOutput=False)
    dxb_ext = nc.declare_dram_parameter("dxb", [18, 1], f32, isOutput=False)
    one18_ext = nc.declare_dram_parameter("one18", [18, 1], f32, isOutput=False)
    out_ext = nc.declare_dram_parameter("out", [128, HW], f32, isOutput=True)

    offs_dram = nc.dram_tensor("offs_dram", [64, HW], fp16)
    # rows = img; cols = (kk, half, j, n) so j-runs are contiguous per (kk, h)
    amap_dram = nc.dram_tensor("amap_dram", [2, KK * 2 * KK * HALF], fp16)

    with tile.TileContext(nc) as tc, ExitStack() as ctx:
        pool = ctx.enter_context(tc.tile_pool(name="sbuf", bufs=1))
        apool = ctx.enter_context(tc.tile_pool(name="amaps", bufs=1))
        tpool = ctx.enter_context(tc.tile_pool(name="tents", bufs=1))
        abuf = ctx.enter_context(tc.tile_pool(name="areps", bufs=3))
        dbuf = ctx.enter_context(tc.tile_pool(name="dstream", bufs=2))
        ppool = ctx.enter_context(tc.tile_pool(name="psum", bufs=1, space="PSUM"))
        pbc = ctx.enter_context(tc.tile_pool(name="psumbc", bufs=2, space="PSUM"))

        # ---- inputs ----
        xp = pool.tile([128, HP * WP], fp16)
        nc.gpsimd.dma_start(xp[:], xp_ext[:])
        xp3 = xp[:].rearrange("p (r c) -> p r c", c=WP)  # [128, 68, 68]

        # weights live on BOTH partition halves (matmul lhsT must share the
        # rhs base partition; img1 rhs starts at partition 64)
        woff = pool.tile([128, KK * 18], fp16)
        wdcn = pool.tile([128, KK * COUT], fp16)
        for hh in range(2):
            nc.sync.dma_start(
                woff[hh * 64 : (hh + 1) * 64, :].rearrange("c (k m) -> c k m", m=18),
                woff_ext[:].rearrange("k c m -> c k m"),
            )
            nc.sync.dma_start(
                wdcn[hh * 64 : (hh + 1) * 64, :].rearrange("c (k m) -> c k m", m=COUT),
                wdcn_ext[:].rearrange("k c m -> c k m"),
            )
        boff = pool.tile([64, 1], f32)
        nc.sync.dma_start(boff[:], boff_ext[:])
        bdcn = pool.tile([128, 1], f32)
        nc.sync.dma_start(bdcn[:], bdcn_ext[:])
        ones2 = pool.tile([18, KK * 128], fp16)
        nc.sync.dma_start(ones2[:], ones2_ext[:])
        dyb = pool.tile([18, 1], f32)
        nc.sync.dma_start(dyb[:], dyb_ext[:])
        dxb = pool.tile([18, 1], f32)
        nc.sync.dma_start(dxb[:], dxb_ext[:])
        one18 = pool.tile([18, 1], f32)
        nc.sync.dma_start(one18[:], one18_ext[:])

        offs_sb = pool.tile([64, HW], fp16)
        out_sb = pool.tile([128, HW], f32)

        # ---- S1/S2: offset conv in 1024-col chunks on rotating PSUM tiles
        # (bc pool), so h1 chunks can be emitted during the h0 MAC ----
        def emit_offconv_chunk(q):
            ps = pbc.tile([128, 1024], f32, tag="bc")
            for img in range(IMG_PER_CORE):
                for sub in range(2):
                    tg = q * 2 + sub
                    for kk in range(KK):
                        ky, kx = kk // 3, kk % 3
                        rhs = xp3[
                            img * 64 : (img + 1) * 64,
                            (PADR - 1 + ky + 8 * tg) : (PADR - 1 + ky + 8 * tg + 8),
                            (PADC - 1 + kx) : (PADC - 1 + kx + W),
                        ]
                        nc.tensor.matmul(
                            ps[img * 32 : img * 32 + 18, sub * 512 : (sub + 1) * 512],
                            woff[img * 64 : (img + 1) * 64, kk * 18 : (kk + 1) * 18],
                            rhs,
                            start=(kk == 0),
                            stop=(kk == KK - 1),
                        )
            qs = slice(q * 1024, (q + 1) * 1024)
            nc.scalar.activation(
                out=offs_sb[:, qs], in_=ps[0:64, :],
                func=AF.Identity, bias=boff[:],
            )
            nc.sync.dma_start(offs_dram[:, qs], offs_sb[:, qs])

        emit_offconv_chunk(0)
        emit_offconv_chunk(1)

        # offs_dram rows = img*32 + 2*kk + axis
        offs2 = offs_dram[:].rearrange("(i m) n -> i m n", i=2)  # [2, 32, HW]
        # amap_dram views: [2, kk, h, j, n] and [2, kk, h, j*n]
        ad5 = amap_dram[:].rearrange(
            "p (k h j n) -> p k h j n", k=KK, h=2, j=KK
        )
        ad4 = amap_dram[:].rearrange(
            "p (k h m) -> p k h m", k=KK, h=2
        )  # [2, 9, 2, KK*HALF]

        # ---- main loop: tents -> hybrid broadcast -> DVE MAC -> matmuls ----
        amap_tiles = {}

        def emit_tents_c(kk, c):
            # tents for tap kk, compact layout [18, HALF] per half: row img*9+j
            if True:
                amap_t = apool.tile([18, HALF], fp16, tag=f"amap{kk}_{c}")
                amap_tiles[(kk, c)] = amap_t
                cs = slice(c * HALF, (c + 1) * HALF)
                tin_y = tpool.tile([18, HALF], fp16, tag="tiny")
                src = offs2[:, 2 * kk : 2 * kk + 1, cs].broadcast_to(
                    [2, KK, HALF]
                )
                nc.sync.dma_start(tin_y[:], src)
                tin_x = tpool.tile([18, HALF], fp16, tag="tinx")
                src = offs2[:, 2 * kk + 1 : 2 * kk + 2, cs].broadcast_to(
                    [2, KK, HALF]
                )
                nc.sync.dma_start(tin_x[:], src)
                uy = tpool.tile([18, HALF], fp16, tag="uy")
                nc.scalar.activation(
                    out=uy[:], in_=tin_y[:], func=AF.Abs, bias=dyb[:]
                )
                ux = tpool.tile([18, HALF], fp16, tag="ux")
                nc.scalar.activation(
                    out=ux[:], in_=tin_x[:], func=AF.Abs, bias=dxb[:]
                )
                # min(u,1) - 1 = -relu(1-u); the negations cancel in the
                # product, so amap = relu(1-uy) * relu(1-ux) exactly.
                ty = tpool.tile([18, HALF], fp16, tag="ty")
                nc.vector.tensor_scalar(
                    out=ty[:], in0=uy[:], scalar1=1.0, scalar2=1.0,
                    op0=AOp.min, op1=AOp.subtract,
                )
                tx = tpool.tile([18, HALF], fp16, tag="tx")
                nc.vector.tensor_scalar(
                    out=tx[:], in0=ux[:], scalar1=1.0, scalar2=1.0,
                    op0=AOp.min, op1=AOp.subtract,
                )
                nc.vector.tensor_tensor(
                    out=amap_t[:], in0=ty[:], in1=tx[:], op=AOp.mult
                )
                for j in DMA_JS:
                    nc.sync.dma_start(
                        ad5[:, kk, c, j, :],
                        amap_t[j : j + KK + 1 : KK, :],
                    )

        emit_tents_c(0, 0)
        for h in range(2):
            ps_main = ppool.tile([128, HALF], f32, tag="big")
            for kk in range(KK):
                ky, kx = kk // 3, kk % 3
                amap_t = amap_tiles[(kk, h)]

                # SWDGE broadcast for js {0,1} as one pair (issued at block
                # head so it is not queued behind the gpsimd tensor ops)
                pairt = abuf.tile([128, 2 * HALF], fp16, tag="pair")
                src = (
                    ad4[:, kk, h, 0 : 2 * HALF]
                    .unsqueeze(1)
                    .broadcast_to([2, 64, 2 * HALF])
                )
                nc.gpsimd.dma_start(pairt[:], src)
                if h == 0:
                    if kk + 1 < KK:
                        emit_tents_c(kk + 1, 0)
                    if kk >= 3:
                        emit_tents_c(kk - 3, 1)
                if h == 1 and kk == 0:
                    emit_tents_c(6, 1)
                    emit_tents_c(7, 1)
                    emit_tents_c(8, 1)

                cols = dbuf.tile([128, HALF], fp16, tag="cols")
                for ji, j in enumerate((2, 3, 4, 5, 6, 7, 8, 0, 1)):
                    dy, dx = j // 3 - 1, j % 3 - 1
                    if j in (0, 1):
                        arep_ap = pairt[:, j * HALF : (j + 1) * HALF]
                    else:
                        # PE ones-mask broadcast into PSUM + ACT copy
                        arep = dbuf.tile([128, HALF], fp16, tag="arep")
                        for q in range(2):
                            psb = pbc.tile([128, 1024], f32, tag="bc")
                            for cc in range(2):
                                n0 = q * 1024 + cc * 512
                                nc.tensor.matmul(
                                    psb[:, cc * 512 : (cc + 1) * 512],
                                    ones2[:, j * 128 : (j + 1) * 128],
                                    amap_t[:, n0 : n0 + 512],
                                    start=True,
                                    stop=True,
                                )
                            nc.scalar.activation(
                                out=arep[:, q * 1024 : (q + 1) * 1024],
                                in_=psb[:], func=AF.Identity,
                            )
                        arep_ap = arep[:]
                    xwin = xp3[
                        :,
                        (PADR - 1 + ky + dy + 32 * h) : (PADR - 1 + ky + dy + 32 * h + 32),
                        (PADC - 1 + kx + dx) : (PADC - 1 + kx + dx + W),
                    ]  # [128, 32, 64]
                    if ji == 0:
                        nc.vector.tensor_tensor(
                            out=cols[:].rearrange("p (a b) -> p a b", b=W),
                            in0=xwin,
                            in1=arep_ap.rearrange("p (a b) -> p a b", b=W),
                            op=AOp.mult,
                        )
                    else:
                        prod = dbuf.tile([128, HALF], fp16, tag="prod")
                        nc.vector.tensor_tensor(
                            out=prod[:].rearrange("p (a b) -> p a b", b=W),
                            in0=xwin,
                            in1=arep_ap.rearrange("p (a b) -> p a b", b=W),
                            op=AOp.mult,
                        )
                        nc.vector.tensor_tensor(
                            out=cols[:], in0=cols[:], in1=prod[:], op=AOp.add
                        )
                for img in range(IMG_PER_CORE):
                    for t in range(4):
                        nc.tensor.matmul(
                            ps_main[
                                img * 64 : (img + 1) * 64, t * 512 : (t + 1) * 512
                            ],
                            wdcn[img * 64 : (img + 1) * 64, kk * COUT : (kk + 1) * COUT],
                            cols[img * 64 : (img + 1) * 64, t * 512 : (t + 1) * 512],
                            start=(kk == 0),
                            stop=(kk == KK - 1),
                        )
                if h == 0 and kk < 2:
                    emit_offconv_chunk(2 + kk)
            hs = slice(h * HALF, (h + 1) * HALF)
            nc.scalar.activation(
                out=out_sb[:, hs], in_=ps_main[:],
                func=AF.Identity, bias=bdcn[:],
            )
            nc.sync.dma_start(out_ext[:, hs], out_sb[:, hs])

    nc.compile()
    return nc


def _host_prep(x, w_off, b_off, w_dcn, b_dcn):
    """Per-core input maps. numpy layout/dtype prep only."""
    fp16 = np.float16
    x = np.asarray(x, dtype=np.float32)
    w_off = np.asarray(w_off, dtype=np.float32)
    b_off = np.asarray(b_off, dtype=np.float32)
    w_dcn = np.asarray(w_dcn, dtype=np.float32)
    b_dcn = np.asarray(b_dcn, dtype=np.float32)

    # lhsT per tap: [KK, CIN, M]
    woff_l = np.ascontiguousarray(
        w_off.transpose(2, 3, 1, 0).reshape(KK, CIN, 18)
    ).astype(fp16)
    wdcn_l = np.ascontiguousarray(
        w_dcn.transpose(2, 3, 1, 0).reshape(KK, CIN, COUT)
    ).astype(fp16)

    boff_rep = np.zeros((64, 1), np.float32)
    for img in range(IMG_PER_CORE):
        boff_rep[img * 32 : img * 32 + 18, 0] = b_off
    bdcn_rep = np.tile(b_dcn, IMG_PER_CORE).reshape(128, 1).astype(np.float32)

    ones2 = np.zeros((18, KK * 128), fp16)
    dyb = np.zeros((18, 1), np.float32)
    dxb = np.zeros((18, 1), np.float32)
    for j in range(KK):
        for img in range(2):
            r = img * KK + j
            ones2[r, j * 128 + img * 64 : j * 128 + (img + 1) * 64] = 1.0
            dyb[r, 0] = -(j // 3 - 1)
            dxb[r, 0] = -(j % 3 - 1)
    one18 = np.ones((18, 1), np.float32)

    shared = {
        "woff": woff_l,
        "wdcn": wdcn_l,
        "boff": boff_rep,
        "bdcn": bdcn_rep,
        "ones2": ones2,
        "dyb": dyb,
        "dxb": dxb,
        "one18": one18,
    }
    in_maps = []
    for core in range(N_CORES):
        imgs = x[core * IMG_PER_CORE : (core + 1) * IMG_PER_CORE]
        xp = np.zeros((IMG_PER_CORE, CIN, HP, WP), np.float32)
        xp[:, :, PADR : PADR + H, PADC : PADC + W] = imgs
        m = {"xp": xp.reshape(128, HP * WP).astype(fp16)}
        m.update(shared)
        in_maps.append(m)
    return in_maps


def kernel(x, w_off, b_off, w_dcn, b_dcn, _trace=False):
    from concourse.bass_utils import run_bass_kernel_spmd

    if "nc" not in _cache:
        _cache["nc"] = _build_program()
    nc = _cache["nc"]

    in_maps = _host_prep(x, w_off, b_off, w_dcn, b_dcn)
    res = run_bass_kernel_spmd(nc, in_maps, list(range(N_CORES)), trace=_trace)
    _cache["last_result"] = res

    out = np.empty((B, COUT, H, W), np.float32)
    for core in range(N_CORES):
        o = np.asarray(res.results[core]["out"], dtype=np.float32)
        out[core * IMG_PER_CORE : (core + 1) * IMG_PER_CORE] = o.reshape(
            IMG_PER_CORE, COUT, H, W
        )
    return out


# revision 59
# speedup vs baseline: 1.0914x; 1.0007x over previous
"""Trainium2 Bass kernel for nn_DeformConv2d (B=16, Cin=Cout=64, H=W=64, K=3).

Strategy (data-parallel over batch, 2 images per core on 8 cores):
  1. PE: offset conv (9 accumulating matmuls per image, K=64, M=18),
     processed in two 2048-column halves.
  2. ACT: bilinear "tent" coefficients tent(delta - D) = relu(1 - |delta - D|)
     via Abs/Relu activations in a compact layout amap[img*9 + j, n] per tap
     (j = window offset); DVE multiplies the y/x tent factors.
  3. Hybrid broadcast of the per-(tap, j) coefficient row pair across the
     128 channel partitions:
       - js {0,1,8}: SWDGE DMA (gpsimd-issued; spreads over all 16 SDMA
         engines) replicating from a DRAM copy of the coefficient maps.
       - js {2..7}: PE ones-mask matmul ([18,128] selector lhsT) into PSUM,
         then ACT copies PSUM -> SBUF fp16.
     This splits the replication load across three otherwise-idle paths so
     the DVE multiply-accumulate stays the critical path.
  4. DVE: 81-term shifted-window multiply-accumulate builds the im2col
     tensor cols per tap (9 mults + 8 adds of [128, 2048] fp16 per tap/half).
  5. PE: main conv = 9 accumulating matmuls (K=64, M=64) per image per half
     into a [128, 2048] f32 PSUM strip; ACT adds bias and writes f32 out.

On-chip compute is fp16 (DVE 2x mode; PSUM accumulates in f32).
kernel() accepts FULL inputs and returns the FULL [16,64,64,64] output.
"""

import numpy as np
from contextlib import ExitStack

N_CORES = 8
B, CIN, COUT, H, W = 16, 64, 64, 64, 64
KK = 9  # 3x3 taps
HW = H * W  # 4096
PADR, PADC = 2, 2
HP, WP = H + 2 * PADR, W + 2 * PADC  # 68, 68
IMG_PER_CORE = B // N_CORES  # 2
HALF = HW // 2

DMA_JS = (0, 1)  # js broadcast via SWDGE DMA (as one pair)
_cache = {}
DEBUG = False


def _build_program():
    import concourse.bass as bass  # noqa: F401
    import concourse.mybir as mybir
    import concourse.tile as tile
    from concourse import bacc

    fp16 = mybir.dt.float16
    f32 = mybir.dt.float32
    AOp = mybir.AluOpType
    AF = mybir.ActivationFunctionType

    nc = bacc.Bacc("TRN2", target_bir_lowering=False, debug=False,
                   num_devices=N_CORES)

    xp_ext = nc.declare_dram_parameter("xp", [128, HP * WP], fp16, isOutput=False)
    woff_ext = nc.declare_dram_parameter("woff", [KK, CIN, 18], fp16, isOutput=False)
    wdcn_ext = nc.declare_dram_parameter("wdcn", [KK, CIN, COUT], fp16, isOutput=False)
    boff_ext = nc.declare_dram_parameter("boff", [64, 1], f32, isOutput=False)
    bdcn_ext = nc.declare_dram_parameter("bdcn", [128, 1], f32, isOutput=False)
    ones2_ext = nc.declare_dram_parameter("ones2", [18, KK * 128], fp16, isOutput=False)
    # per-row tent window offsets: biases -D for |delta - D|
    dyb_ext = nc.declare_dram_parameter("dyb", [18, 1], f32, isOutput=False)
    dxb_ext = nc.declare_dram_parameter("dxb", [18, 1], f32, isOutput=False)
    one18_ext = nc.declare_dram_parameter("one18", [18, 1], f32, isOutput=False)
    out_ext = nc.declare_dram_parameter("out", [128, HW], f32, isOutput=True)

    offs_dram = nc.dram_tensor("offs_dram", [64, HW], fp16)
    # rows = img; cols = (kk, half, j, n) so j-runs are contiguous per (kk, h)
    amap_dram = nc.dram_tensor("amap_dram", [2, KK * 2 * KK * HALF], fp16)

    with tile.TileContext(nc) as tc, ExitStack() as ctx:
        pool = ctx.enter_context(tc.tile_pool(name="sbuf", bufs=1))
        apool = ctx.enter_context(tc.tile_pool(name="amaps", bufs=1))
        tpool = ctx.enter_context(tc.tile_pool(name="tents", bufs=1))
        abuf = ctx.enter_context(tc.tile_pool(name="areps", bufs=3))
        dbuf = ctx.enter_context(tc.tile_pool(name="dstream", bufs=2))
        ppool = ctx.enter_context(tc.tile_pool(name="psum", bufs=1, space="PSUM"))
        pbc = ctx.enter_context(tc.tile_pool(name="psumbc", bufs=2, space="PSUM"))

        # ---- inputs ----
        xp = pool.tile([128, HP * WP], fp16)
        nc.gpsimd.dma_start(xp[:], xp_ext[:])
        xp3 = xp[:].rearrange("p (r c) -> p r c", c=WP)  # [128, 68, 68]

        # weights live on BOTH partition halves (matmul lhsT must share the
        # rhs base partition; img1 rhs starts at partition 64)
        woff = pool.tile([128, KK * 18], fp16)
        wdcn = pool.tile([128, KK * COUT], fp16)
        for hh in range(2):
            nc.sync.dma_start(
                woff[hh * 64 : (hh + 1) * 64, :].rearrange("c (k m) -> c k m", m=18),
                woff_ext[:].rearrange("k c m -> c k m"),
            )
            nc.sync.dma_start(
                wdcn[hh * 64 : (hh + 1) * 64, :].rearrange("c (k m) -> c k m", m=COUT),
                wdcn_ext[:].rearrange("k c m -> c k m"),
            )
        boff = pool.tile([64, 1], f32)
        nc.sync.dma_start(boff[:], boff_ext[:])
        bdcn = pool.tile([128, 1], f32)
        nc.sync.dma_start(bdcn[:], bdcn_ext[:])
        ones2 = pool.tile([18, KK * 128], fp16)
        nc.sync.dma_start(ones2[:], ones2_ext[:])
        dyb = pool.tile([18, 1], f32)
        nc.sync.dma_start(dyb[:], dyb_ext[:])
        dxb = pool.tile([18, 1], f32)
        nc.sync.dma_start(dxb[:], dxb_ext[:])
        one18 = pool.tile([18, 1], f32)
        nc.sync.dma_start(one18[:], one18_ext[:])

        offs_sb = pool.tile([64, HW], fp16)
        out_sb = pool.tile([128, HW], f32)

        # ---- S1/S2: offset conv in 1024-col chunks on rotating PSUM tiles
        # (bc pool), so h1 chunks can be emitted during the h0 MAC ----
        def emit_offconv_chunk(q):
            ps = pbc.tile([128, 1024], f32, tag="bc")
            for img in range(IMG_PER_CORE):
                for sub in range(2):
                    tg = q * 2 + sub
                    for kk in range(KK):
                        ky, kx = kk // 3, kk % 3
                        rhs = xp3[
                            img * 64 : (img + 1) * 64,
                            (PADR - 1 + ky + 8 * tg) : (PADR - 1 + ky + 8 * tg + 8),
                            (PADC - 1 + kx) : (PADC - 1 + kx + W),
                        ]
                        nc.tensor.matmul(
                            ps[img * 32 : img * 32 + 18, sub * 512 : (sub + 1) * 512],
                            woff[img * 64 : (img + 1) * 64, kk * 18 : (kk + 1) * 18],
                            rhs,
                            start=(kk == 0),
                            stop=(kk == KK - 1),
                        )
            qs = slice(q * 1024, (q + 1) * 1024)
            nc.scalar.activation(
                out=offs_sb[:, qs], in_=ps[0:64, :],
                func=AF.Identity, bias=boff[:],
            )
            nc.sync.dma_start(offs_dram[:, qs], offs_sb[:, qs])

        emit_offconv_chunk(0)
        emit_offconv_chunk(1)

        # offs_dram rows = img*32 + 2*kk + axis
        offs2 = offs_dram[:].rearrange("(i m) n -> i m n", i=2)  # [2, 32, HW]
        # amap_dram views: [2, kk, h, j, n] and [2, kk, h, j*n]
        ad5 = amap_dram[:].rearrange(
            "p (k h j n) -> p k h j n", k=KK, h=2, j=KK
        )
        ad4 = amap_dram[:].rearrange(
            "p (k h m) -> p k h m", k=KK, h=2
        )  # [2, 9, 2, KK*HALF]

        # ---- main loop: tents -> hybrid broadcast -> DVE MAC -> matmuls ----
        amap_tiles = {}

        def emit_tents_c(kk, c):
            # tents for tap kk, compact layout [18, HALF] per half: row img*9+j
            if True:
                amap_t = apool.tile([18, HALF], fp16, tag=f"amap{kk}_{c}")
                amap_tiles[(kk, c)] = amap_t
                cs = slice(c * HALF, (c + 1) * HALF)
                tin_y = tpool.tile([18, HALF], fp16, tag="tiny")
                src = offs2[:, 2 * kk : 2 * kk + 1, cs].broadcast_to(
                    [2, KK, HALF]
                )
                nc.sync.dma_start(tin_y[:], src)
                tin_x = tpool.tile([18, HALF], fp16, tag="tinx")
                src = offs2[:, 2 * kk + 1 : 2 * kk + 2, cs].broadcast_to(
                    [2, KK, HALF]
                )
                nc.sync.dma_start(tin_x[:], src)
                uy = tpool.tile([18, HALF], fp16, tag="uy")
                nc.scalar.activation(
                    out=uy[:], in_=tin_y[:], func=AF.Abs, bias=dyb[:]
                )
                ux = tpool.tile([18, HALF], fp16, tag="ux")
                nc.scalar.activation(
                    out=ux[:], in_=tin_x[:], func=AF.Abs, bias=dxb[:]
                )
                # min(u,1) - 1 = -relu(1-u); the negations cancel in the
                # product, so amap = relu(1-uy) * relu(1-ux) exactly.
                ty = tpool.tile([18, HALF], fp16, tag="ty")
                nc.vector.tensor_scalar(
                    out=ty[:], in0=uy[:], scalar1=1.0, scalar2=1.0,
                    op0=AOp.min, op1=AOp.subtract,
                )
                tx = tpool.tile([18, HALF], fp16, tag="tx")
                nc.vector.tensor_scalar(
                    out=tx[:], in0=ux[:], scalar1=1.0, scalar2=1.0,
                    op0=AOp.min, op1=AOp.subtract,
                )
                nc.vector.tensor_tensor(
                    out=amap_t[:], in0=ty[:], in1=tx[:], op=AOp.mult
                )
                for j in DMA_JS:
                    nc.sync.dma_start(
                        ad5[:, kk, c, j, :],
                        amap_t[j : j + KK + 1 : KK, :],
                    )

        # SWDGE broadcast for js {0,1} as one pair, prefetched one tap
        # block ahead of its consumption
        pair_tiles = {}

        def issue_pair(ph, pkk):
            pairt = abuf.tile([128, 2 * HALF], fp16, tag="pair")
            pair_tiles[(ph, pkk)] = pairt
            src = (
                ad4[:, pkk, ph, 0 : 2 * HALF]
                .unsqueeze(1)
                .broadcast_to([2, 64, 2 * HALF])
            )
            nc.gpsimd.dma_start(pairt[:], src)

        emit_tents_c(0, 0)
        issue_pair(0, 0)
        for h in range(2):
            ps_main = ppool.tile([128, HALF], f32, tag="big")
            for kk in range(KK):
                ky, kx = kk // 3, kk % 3
                amap_t = amap_tiles[(kk, h)]
                if h == 0:
                    if kk + 1 < KK:
                        emit_tents_c(kk + 1, 0)
                    if kk >= 3:
                        emit_tents_c(kk - 3, 1)
                if h == 1 and kk == 0:
                    emit_tents_c(6, 1)
                    emit_tents_c(7, 1)
                    emit_tents_c(8, 1)
                if kk + 1 < KK:
                    issue_pair(h, kk + 1)
                elif h == 0:
                    issue_pair(1, 0)
                pairt = pair_tiles[(h, kk)]

                cols = dbuf.tile([128, HALF], fp16, tag="cols")
                for ji, j in enumerate((2, 3, 4, 5, 6, 7, 8, 0, 1)):
                    dy, dx = j // 3 - 1, j % 3 - 1
                    if j in (0, 1):
                        arep_ap = pairt[:, j * HALF : (j + 1) * HALF]
                    else:
                        # PE ones-mask broadcast into PSUM + ACT copy
                        arep = dbuf.tile([128, HALF], fp16, tag="arep")
                        for q in range(2):
                            psb = pbc.tile([128, 1024], f32, tag="bc")
                            for cc in range(2):
                                n0 = q * 1024 + cc * 512
                                nc.tensor.matmul(
                                    psb[:, cc * 512 : (cc + 1) * 512],
                                    ones2[:, j * 128 : (j + 1) * 128],
                                    amap_t[:, n0 : n0 + 512],
                                    start=True,
                                    stop=True,
                                )
                            nc.scalar.activation(
                                out=arep[:, q * 1024 : (q + 1) * 1024],
                                in_=psb[:], func=AF.Identity,
                            )
                        arep_ap = arep[:]
                    xwin = xp3[
                        :,
                        (PADR - 1 + ky + dy + 32 * h) : (PADR - 1 + ky + dy + 32 * h + 32),
                        (PADC - 1 + kx + dx) : (PADC - 1 + kx + dx + W),
                    ]  # [128, 32, 64]
                    if ji == 0:
                        nc.vector.tensor_tensor(
                            out=cols[:].rearrange("p (a b) -> p a b", b=W),
                            in0=xwin,
                            in1=arep_ap.rearrange("p (a b) -> p a b", b=W),
                            op=AOp.mult,
                        )
                    else:
                        prod = dbuf.tile([128, HALF], fp16, tag="prod")
                        nc.vector.tensor_tensor(
                            out=prod[:].rearrange("p (a b) -> p a b", b=W),
                            in0=xwin,
                            in1=arep_ap.rearrange("p (a b) -> p a b", b=W),
                            op=AOp.mult,
                        )
                        nc.vector.tensor_tensor(
                            out=cols[:], in0=cols[:], in1=prod[:], op=AOp.add
                        )
                for img in range(IMG_PER_CORE):
                    for t in range(4):
                        nc.tensor.matmul(
                            ps_main[
                                img * 64 : (img + 1) * 64, t * 512 : (t + 1) * 512
                            ],
                            wdcn[img * 64 : (img + 1) * 64, kk * COUT : (kk + 1) * COUT],
                            cols[img * 64 : (img + 1) * 64, t * 512 : (t + 1) * 512],
                            start=(kk == 0),
                            stop=(kk == KK - 1),
                        )
                if h == 0 and kk < 2:
                    emit_offconv_chunk(2 + kk)
            hs = slice(h * HALF, (h + 1) * HALF)
            nc.scalar.activation(
                out=out_sb[:, hs], in_=ps_main[:],
                func=AF.Identity, bias=bdcn[:],
            )
            nc.sync.dma_start(out_ext[:, hs], out_sb[:, hs])

    nc.compile()
    return nc


def _host_prep(x, w_off, b_off, w_dcn, b_dcn):
    """Per-core input maps. numpy layout/dtype prep only."""
    fp16 = np.float16
    x = np.asarray(x, dtype=np.float32)
    w_off = np.asarray(w_off, dtype=np.float32)
    b_off = np.asarray(b_off, dtype=np.float32)
    w_dcn = np.asarray(w_dcn, dtype=np.float32)
    b_dcn = np.asarray(b_dcn, dtype=np.float32)

    # lhsT per tap: [KK, CIN, M]
    woff_l = np.ascontiguousarray(
        w_off.transpose(2, 3, 1, 0).reshape(KK, CIN, 18)
    ).astype(fp16)
    wdcn_l = np.ascontiguousarray(
        w_dcn.transpose(2, 3, 1, 0).reshape(KK, CIN, COUT)
    ).astype(fp16)

    boff_rep = np.zeros((64, 1), np.float32)
    for img in range(IMG_PER_CORE):
        boff_rep[img * 32 : img * 32 + 18, 0] = b_off
    bdcn_rep = np.tile(b_dcn, IMG_PER_CORE).reshape(128, 1).astype(np.float32)

    ones2 = np.zeros((18, KK * 128), fp16)
    dyb = np.zeros((18, 1), np.float32)
    dxb = np.zeros((18, 1), np.float32)
    for j in range(KK):
        for img in range(2):
            r = img * KK + j
            ones2[r, j * 128 + img * 64 : j * 128 + (img + 1) * 64] = 1.0
            dyb[r, 0] = -(j // 3 - 1)
            dxb[r, 0] = -(j % 3 - 1)
    one18 = np.ones((18, 1), np.float32)

    shared = {
        "woff": woff_l,
        "wdcn": wdcn_l,
        "boff": boff_rep,
        "bdcn": bdcn_rep,
        "ones2": ones2,
        "dyb": dyb,
        "dxb": dxb,
        "one18": one18,
    }
    in_maps = []
    for core in range(N_CORES):
        imgs = x[core * IMG_PER_CORE : (core + 1) * IMG_PER_CORE]
        xp = np.zeros((IMG_PER_CORE, CIN, HP, WP), np.float32)
        xp[:, :, PADR : PADR + H, PADC : PADC + W] = imgs
        m = {"xp": xp.reshape(128, HP * WP).astype(fp16)}
        m.update(shared)
        in_maps.append(m)
    return in_maps


def kernel(x, w_off, b_off, w_dcn, b_dcn, _trace=False):
    from concourse.bass_utils import run_bass_kernel_spmd

    if "nc" not in _cache:
        _cache["nc"] = _build_program()
    nc = _cache["nc"]

    in_maps = _host_prep(x, w_off, b_off, w_dcn, b_dcn)
    res = run_bass_kernel_spmd(nc, in_maps, list(range(N_CORES)), trace=_trace)
    _cache["last_result"] = res

    out = np.empty((B, COUT, H, W), np.float32)
    for core in range(N_CORES):
        o = np.asarray(res.results[core]["out"], dtype=np.float32)
        out[core * IMG_PER_CORE : (core + 1) * IMG_PER_CORE] = o.reshape(
            IMG_PER_CORE, COUT, H, W
        )
    return out
